# revision 27
# baseline (speedup 1.0000x reference)
"""Trainium2 Bass kernel for nn_CQRNLayer (quasi-recurrent conv layer).

Computation (per the reference):
  Y = Conv2d(C=64 -> 3H=192, kernel (1,9), pad (0,4)) over (S*B, C, 1, N) + bias
  Z, F, O = split(Y); Z = elu(Z); F = sigmoid(F); O = sigmoid(O)
  C_t = F_t * Z_t + (1 - F_t) * C_{t-1}   (scan over S, C_{-1} = hidden)
  Hout = O * C_seq ; outputs (Hout, C_seq[-1:])

Sharding: data-parallel over batch (B=16 -> 2 per core on 8 cores).

Kernel strategy (per core):
  * Conv as K=128, M=128, free=512 matmuls: the 9 taps are packed in pairs
    along the contraction dim.  SBUF holds X twice: partitions 0:64 with the
    n-axis halo-padded (shift 4), partitions 64:128 shifted by 3; a tap pair
    (2p, 2p+1) then reads one shifted window of both halves.  Tap 8 rides a
    zero-padded second half of the weights.  The 192 output channels are
    covered by two M=128 groups: [Z;F] (ch 0:128) and [F;O] (ch 64:192,
    F computed twice) - uniform full-array matmuls, float32r at full PE rate.
  * Weights pre-packed on host to [2, 5, 128, 128] (group, pair, K, M).
  * ScalarE evacuates PSUM applying exp/sigmoid (+conv bias) directly into
    an s-innermost layout [128=(b,h), n, s], using per-b partition-shifted
    passes.
  * elu(z)+1 = max(z + 1, min(exp(z), 1)) -> DVE scalar_tensor_tensor reading
    the conv PSUM directly (no relu pass).
  * The recurrence runs on the DVE hardware scan (tensor_tensor_scan) along
    the free dim: chains of length SC per n, separated by reset columns
    (G=0, FZ=carry) so one instruction scans a whole [128, n*(SC+1)] tile.
  * Hout = O * C on GpSimd; DMA out per chunk.
"""

import os
import sys

import numpy as np

for _p in ("/opt/trn_rl_repo", "/root/.axon_site/_ro/trn_rl_repo"):
    if os.path.isdir(_p) and _p not in sys.path:
        sys.path.append(_p)

import concourse.bass as bass
import concourse.bacc as bacc
import concourse.mybir as mybir
import concourse.tile as tile

FP = mybir.dt.float32
AF = mybir.ActivationFunctionType
OP = mybir.AluOpType

S, B, C, N, H = 256, 16, 64, 128, 64
NCORES = 8
B_LOC = B // NCORES  # 2
SC = 16              # seq-len macro chunk
NP_PAD = 136         # 128 + 2*4 halo for the 9-tap conv
NP_IN = NP_PAD + 1   # host-padded n extent (both shifted reads from one array)

MM_DT = mybir.dt.float32r  # conv matmul operand dtype (full PE rate, ~fp32)

LAST_RESULTS = None  # BassKernelResults of the most recent run (for test.py)


def build_program(s_total: int = S, reps: int = 1):
    nchunks = s_total // SC
    nc = bacc.Bacc(
        "TRN2", target_bir_lowering=False, debug=False, num_devices=NCORES
    )

    xm = nc.declare_dram_parameter("xm", [2 * C, s_total, B_LOC, NP_PAD], MM_DT, isOutput=False)
    wl = nc.declare_dram_parameter("wl", [2, 5, 128, 128], MM_DT, isOutput=False)
    bias = nc.declare_dram_parameter("bias", [128, 4], FP, isOutput=False)
    h0 = nc.declare_dram_parameter("h0", [128, N], FP, isOutput=False)
    hout = nc.declare_dram_parameter("hout", [s_total, B_LOC, H, N], FP, isOutput=True)
    clast = nc.declare_dram_parameter("clast", [128, N], FP, isOutput=True)

    with tile.TileContext(nc) as tc:
        with (
            tc.tile_pool(name="constp", bufs=1) as constp,
            tc.tile_pool(name="x2p", bufs=2) as x2p,
            tc.tile_pool(name="actp", bufs=2) as actp,
            tc.tile_pool(name="scanp", bufs=2) as scanp,
            tc.tile_pool(name="psump", bufs=2, space="PSUM") as psump,
        ):
            w_sb = constp.tile([128, 2, 5, 128], MM_DT)
            nc.sync.dma_start(w_sb[:, :, :, :], wl[:, :, :, :].transpose([2, 0, 1, 3]))
            bias_sb = constp.tile([128, 4], FP)
            nc.sync.dma_start(bias_sb[:, :], bias[:, :])
            hid_sb = constp.tile([128, N], FP)
            nc.sync.dma_start(hid_sb[:, :], h0[:, :])

            for rep in range(reps):
                prev_c = None
                for ci in range(nchunks):
                    s0 = ci * SC
                    # ---- X staging: two n-shifted copies on the partition halves
                    # (host pre-shifts; each DMA is one long contiguous run per
                    # partition)
                    x2 = x2p.tile([128, SC, B_LOC, NP_PAD], MM_DT, tag="x2")
                    nc.sync.dma_start(x2[:, :, :, :], xm[:, s0:s0 + SC, :, :])

                    # ---- conv + evacuation, one (grp, b) unit at a time,
                    # cycling two 4-bank PSUM slots so the PE never idles.
                    # psum layout [128ch, s, n]; gate tiles [(b,h), n, s].
                    expz = actp.tile([128, N, SC], FP, tag="expz")
                    f_t = actp.tile([128, N, SC], FP, tag="f_t")
                    o_t = actp.tile([128, N, SC], FP, tag="o_t")
                    q_t = actp.tile([128, N, SC], FP, tag="q_t", bufs=1)
                    for b in range(B_LOC):
                        hsl = slice(b * 64, b * 64 + 64)
                        for grp in range(2):
                            pt = psump.tile([128, SC, N], FP, tag="ps", name=f"pt{grp}{b}")
                            for mu in range(SC // 4):
                                for p in range(5):
                                    nc.tensor.matmul(
                                        pt[:, mu * 4:(mu + 1) * 4, :],
                                        w_sb[:, grp, p, :],
                                        x2[:, mu * 4:(mu + 1) * 4, b, 2 * p:2 * p + 128],
                                        start=(p == 0),
                                        stop=(p == 4),
                                        skip_group_check=True,
                                    )
                            ptt = pt[:, :, :].transpose([0, 2, 1])  # [ch, n, s]
                            # sigmoid(x) = 0.5*tanh(x/2) + 0.5; tanh shares the
                            # ACT table with exp, so no act-table reloads.
                            if grp == 0:  # [Z; F]
                                nc.scalar.activation(expz[hsl, :, :], ptt[0:64],
                                                     AF.Exp, bias=bias_sb[0:64, 0:1])
                                nc.scalar.activation(f_t[hsl, :, :], ptt[64:128],
                                                     AF.Tanh, scale=0.5,
                                                     bias=bias_sb[64:128, 1:2])
                                # expz <- min(exp, 1) on GpSimd (keeps DVE free)
                                nc.gpsimd.tensor_scalar_min(expz[hsl, :, :],
                                                            expz[hsl, :, :], 1.0)
                                # q = elu(z)+1 = max(z + bz + 1, min(exp(z+bz), 1))
                                nc.vector.scalar_tensor_tensor(
                                    q_t[hsl, :, :], ptt[0:64],
                                    bias_sb[hsl, 3:4], expz[hsl, :, :],
                                    op0=OP.add, op1=OP.max)
                            else:  # [F; O]
                                nc.scalar.activation(o_t[hsl, :, :], ptt[64:128],
                                                     AF.Tanh, scale=0.5,
                                                     bias=bias_sb[64:128, 2:3])

                    # ---- gate prep on DVE (f_t, o_t hold tanh halves:
                    # F = 0.5*f_t + 0.5, O = 0.5*o_t + 0.5)
                    g_t = scanp.tile([128, N, 1 + SC], FP, tag="g_t")
                    nc.gpsimd.memset(g_t[:, :, 0:1], 0.0)
                    nc.vector.tensor_scalar(g_t[:, :, 1:], f_t[:, :, :], -0.5, 0.5,
                                            op0=OP.mult, op1=OP.add)  # G = 1 - F
                    # F = 0.5*th + 0.5 (reconstruct in place on DVE)
                    nc.vector.tensor_scalar(f_t[:, :, :], f_t[:, :, :], 0.5, 0.5,
                                            op0=OP.mult, op1=OP.add)
                    # O = 0.5*th + 0.5 (reconstruct on GpSimd)
                    nc.gpsimd.tensor_scalar(o_t[:, :, :], o_t[:, :, :], 0.5, 0.5,
                                            op0=OP.mult, op1=OP.add)
                    fz = scanp.tile([128, N, 1 + SC], FP, tag="fz")
                    nc.vector.scalar_tensor_tensor(fz[:, :, 1:], q_t[:, :, :], -1.0,
                                                   f_t[:, :, :], op0=OP.add, op1=OP.mult)
                    # carry into the reset column
                    if ci == 0:
                        nc.vector.tensor_copy(fz[:, :, 0], hid_sb[:, :])
                    else:
                        nc.vector.tensor_copy(fz[:, :, 0], prev_c[:, :, SC])

                    # ---- the recurrence: one hardware scan over the whole chunk
                    c_t = scanp.tile([128, N, 1 + SC], FP, tag="c_t")
                    nc.vector.tensor_tensor_scan(
                        c_t.rearrange("p n s -> p (n s)"),
                        g_t.rearrange("p n s -> p (n s)"),
                        fz.rearrange("p n s -> p (n s)"),
                        0.0, op0=OP.mult, op1=OP.add,
                    )
                    prev_c = c_t

                    # ---- Hout = O * C, written in (s, n) layout for the DMA out
                    ho_t = actp.tile([128, SC, N], FP, tag="ho_t")
                    nc.gpsimd.tensor_tensor(ho_t.transpose([0, 2, 1]), o_t[:, :, :],
                                            c_t[:, :, 1:], op=OP.mult)
                    hdst = hout[s0:s0 + SC, :, :, :].transpose([1, 2, 0, 3]) \
                        .rearrange("b h s n -> (b h) s n")
                    nc.sync.dma_start(hdst, ho_t[:, :, :])

                cl_t = actp.tile([128, N], FP, tag="cl_t", bufs=1)
                nc.vector.tensor_copy(cl_t[:, :], prev_c[:, :, SC])
                nc.sync.dma_start(clast[:, :], cl_t[:, :])

    nc.compile()
    return nc


def _mm_np_dtype():
    return mybir.dt.np(MM_DT)


def pack_weights(W: np.ndarray) -> np.ndarray:
    """W [192, 64, 1, 9] -> lhsT tiles [2, 5, 128, 128] (group, tap-pair, K, M).

    Group 0 covers output channels 0:128 ([Z;F]), group 1 covers 64:192
    ([F;O]).  K rows 0:64 hold tap 2p, rows 64:128 hold tap 2p+1 (zeros for
    the nonexistent tap 9)."""
    Wt = np.asarray(W, dtype=np.float32)[:, :, 0, :]  # [192, 64, 9]
    Wl = np.zeros((2, 5, 128, 128), dtype=np.float32)
    for grp, ch0 in ((0, 0), (1, 64)):
        for p in range(5):
            Wl[grp, p, 0:64, :] = Wt[ch0:ch0 + 128, :, 2 * p].T
            if 2 * p + 1 <= 8:
                Wl[grp, p, 64:128, :] = Wt[ch0:ch0 + 128, :, 2 * p + 1].T
    return Wl.astype(_mm_np_dtype())


def pack_bias(b: np.ndarray) -> np.ndarray:
    b = np.asarray(b, dtype=np.float32)
    bias_t = np.zeros((128, 4), dtype=np.float32)
    hh = np.arange(128) % 64
    bias_t[:, 0] = b[hh]                 # z bias (exp pass, partitions = h)
    bias_t[:, 1] = 0.5 * b[64 + hh]      # f bias (tanh half-scale)
    bias_t[:, 2] = 0.5 * b[128 + hh]     # o bias (tanh half-scale)
    bias_t[:, 3] = b[hh] + 1.0           # z bias + 1 (elu max identity)
    return bias_t


def pad_x(input_data: np.ndarray) -> np.ndarray:
    """[S, B, C, N] fp32 -> [2C, S, B, NP_PAD] in MM_DT: rows 0:C hold the
    shift-4 halo layout (taps 2p), rows C:2C the shift-3 layout (taps 2p+1)."""
    s_total, b_, c_, n_ = input_data.shape
    xt = np.asarray(input_data).astype(_mm_np_dtype()).transpose(2, 0, 1, 3)
    xab = np.zeros((2 * c_, s_total, b_, NP_PAD), dtype=_mm_np_dtype())
    xab[0:c_, :, :, 4:4 + n_] = xt
    xab[c_:, :, :, 3:3 + n_] = xt
    return xab


_NC_CACHE = {}


def build_in_maps(inputs: dict) -> list[dict]:
    input_data = np.asarray(inputs["input_data"], dtype=np.float32)
    hidden = np.asarray(inputs["hidden"], dtype=np.float32)
    xm = pad_x(input_data)
    Wl = pack_weights(inputs["W"])
    bias_t = pack_bias(inputs["b"])
    in_maps = []
    for i in range(NCORES):
        bsl = slice(i * B_LOC, (i + 1) * B_LOC)
        in_maps.append({
            "xm": np.ascontiguousarray(xm[:, :, bsl]),
            "bias": bias_t,
            "wl": Wl,
            "h0": np.ascontiguousarray(hidden[bsl]).reshape(128, N),
        })
    return in_maps


def kernel(input_data, hidden, W, b):
    global LAST_RESULTS
    from concourse.bass_utils import run_bass_kernel_spmd

    s_total = np.asarray(input_data).shape[0]
    if s_total not in _NC_CACHE:
        _NC_CACHE[s_total] = build_program(s_total)
    nc = _NC_CACHE[s_total]

    in_maps = build_in_maps({
        "input_data": input_data, "hidden": hidden, "W": W, "b": b,
    })
    res = run_bass_kernel_spmd(nc, in_maps, list(range(NCORES)))
    LAST_RESULTS = res

    houts = [res.results[i]["hout"] for i in range(NCORES)]
    clasts = [res.results[i]["clast"].reshape(B_LOC, H, N) for i in range(NCORES)]
    Hout = np.concatenate(houts, axis=1)
    C_last = np.concatenate(clasts, axis=0)[None]
    return Hout, C_last


# revision 28
# speedup vs baseline: 1.1113x; 1.1113x over previous
"""Trainium2 Bass kernel for nn_CQRNLayer (quasi-recurrent conv layer).

Computation (per the reference):
  Y = Conv2d(C=64 -> 3H=192, kernel (1,9), pad (0,4)) over (S*B, C, 1, N) + bias
  Z, F, O = split(Y); Z = elu(Z); F = sigmoid(F); O = sigmoid(O)
  C_t = F_t * Z_t + (1 - F_t) * C_{t-1}   (scan over S, C_{-1} = hidden)
  Hout = O * C_seq ; outputs (Hout, C_seq[-1:])

Sharding: data-parallel over batch (B=16 -> 2 per core on 8 cores).

Kernel strategy (per core):
  * Conv as K=128, M=128, free=512 matmuls: the 9 taps are packed in pairs
    along the contraction dim.  SBUF holds X twice: partitions 0:64 with the
    n-axis halo-padded (shift 4), partitions 64:128 shifted by 3; a tap pair
    (2p, 2p+1) then reads one shifted window of both halves.  Tap 8 rides a
    zero-padded second half of the weights.  The 192 output channels are
    covered by two M=128 groups: [Z;F] (ch 0:128) and [F;O] (ch 64:192,
    F computed twice) - uniform full-array matmuls, float32r at full PE rate.
  * Weights pre-packed on host to [2, 5, 128, 128] (group, pair, K, M).
  * ScalarE evacuates PSUM applying exp/sigmoid (+conv bias) directly into
    an s-innermost layout [128=(b,h), n, s], using per-b partition-shifted
    passes.
  * elu(z)+1 = max(z + 1, min(exp(z), 1)) -> DVE scalar_tensor_tensor reading
    the conv PSUM directly (no relu pass).
  * The recurrence runs on the DVE hardware scan (tensor_tensor_scan) along
    the free dim: chains of length SC per n, separated by reset columns
    (G=0, FZ=carry) so one instruction scans a whole [128, n*(SC+1)] tile.
  * Hout = O * C on GpSimd; DMA out per chunk.
"""

import os
import sys

import numpy as np

for _p in ("/opt/trn_rl_repo", "/root/.axon_site/_ro/trn_rl_repo"):
    if os.path.isdir(_p) and _p not in sys.path:
        sys.path.append(_p)

import concourse.bass as bass
import concourse.bacc as bacc
import concourse.mybir as mybir
import concourse.tile as tile

FP = mybir.dt.float32
AF = mybir.ActivationFunctionType
OP = mybir.AluOpType

S, B, C, N, H = 256, 16, 64, 128, 64
NCORES = 8
B_LOC = B // NCORES  # 2
SC = 16              # seq-len macro chunk
NP_PAD = 136         # 128 + 2*4 halo for the 9-tap conv
NP_IN = NP_PAD + 1   # host-padded n extent (both shifted reads from one array)

MM_DT = mybir.dt.bfloat16  # conv matmul operand dtype (full PE rate)

LAST_RESULTS = None  # BassKernelResults of the most recent run (for test.py)


def build_program(s_total: int = S, reps: int = 1):
    nchunks = s_total // SC
    nc = bacc.Bacc(
        "TRN2", target_bir_lowering=False, debug=False, num_devices=NCORES
    )

    xm = nc.declare_dram_parameter("xm", [2 * C, s_total, B_LOC, NP_PAD], MM_DT, isOutput=False)
    wl = nc.declare_dram_parameter("wl", [2, 5, 128, 128], MM_DT, isOutput=False)
    bias = nc.declare_dram_parameter("bias", [128, 4], FP, isOutput=False)
    h0 = nc.declare_dram_parameter("h0", [128, N], FP, isOutput=False)
    hout = nc.declare_dram_parameter("hout", [s_total, B_LOC, H, N], FP, isOutput=True)
    clast = nc.declare_dram_parameter("clast", [128, N], FP, isOutput=True)

    with tile.TileContext(nc) as tc:
        with (
            tc.tile_pool(name="constp", bufs=1) as constp,
            tc.tile_pool(name="x2p", bufs=2) as x2p,
            tc.tile_pool(name="actp", bufs=2) as actp,
            tc.tile_pool(name="scanp", bufs=2) as scanp,
            tc.tile_pool(name="psump", bufs=2, space="PSUM") as psump,
        ):
            w_sb = constp.tile([128, 2, 5, 128], MM_DT)
            nc.sync.dma_start(w_sb[:, :, :, :], wl[:, :, :, :].transpose([2, 0, 1, 3]))
            bias_sb = constp.tile([128, 4], FP)
            nc.sync.dma_start(bias_sb[:, :], bias[:, :])
            hid_sb = constp.tile([128, N], FP)
            nc.sync.dma_start(hid_sb[:, :], h0[:, :])

            for rep in range(reps):
                prev_c = None
                for ci in range(nchunks):
                    s0 = ci * SC
                    # ---- X staging: two n-shifted copies on the partition halves
                    # (host pre-shifts; each DMA is one long contiguous run per
                    # partition)
                    x2 = x2p.tile([128, SC, B_LOC, NP_PAD], MM_DT, tag="x2")
                    nc.sync.dma_start(x2[:, :, :, :], xm[:, s0:s0 + SC, :, :])

                    # ---- conv + evacuation, one (grp, b) unit at a time,
                    # cycling two 4-bank PSUM slots so the PE never idles.
                    # psum layout [128ch, s, n]; gate tiles [(b,h), n, s].
                    expz = actp.tile([128, N, SC], FP, tag="expz")
                    f_t = actp.tile([128, N, SC], FP, tag="f_t")
                    o_t = actp.tile([128, N, SC], FP, tag="o_t")
                    q_t = actp.tile([128, N, SC], FP, tag="q_t", bufs=1)
                    for b in range(B_LOC):
                        hsl = slice(b * 64, b * 64 + 64)
                        for grp in range(2):
                            pt = psump.tile([128, SC, N], FP, tag="ps", name=f"pt{grp}{b}")
                            for mu in range(SC // 4):
                                for p in range(5):
                                    nc.tensor.matmul(
                                        pt[:, mu * 4:(mu + 1) * 4, :],
                                        w_sb[:, grp, p, :],
                                        x2[:, mu * 4:(mu + 1) * 4, b, 2 * p:2 * p + 128],
                                        start=(p == 0),
                                        stop=(p == 4),
                                        skip_group_check=True,
                                    )
                            ptt = pt[:, :, :].transpose([0, 2, 1])  # [ch, n, s]
                            # sigmoid(x) = 0.5*tanh(x/2) + 0.5; tanh shares the
                            # ACT table with exp, so no act-table reloads.
                            if grp == 0:  # [Z; F]
                                nc.scalar.activation(expz[hsl, :, :], ptt[0:64],
                                                     AF.Exp, bias=bias_sb[0:64, 0:1])
                                nc.scalar.activation(f_t[hsl, :, :], ptt[64:128],
                                                     AF.Tanh, scale=0.5,
                                                     bias=bias_sb[64:128, 1:2])
                                # expz <- min(exp, 1) on GpSimd (keeps DVE free)
                                nc.gpsimd.tensor_scalar_min(expz[hsl, :, :],
                                                            expz[hsl, :, :], 1.0)
                                # q = elu(z)+1 = max(z + bz + 1, min(exp(z+bz), 1))
                                nc.vector.scalar_tensor_tensor(
                                    q_t[hsl, :, :], ptt[0:64],
                                    bias_sb[hsl, 3:4], expz[hsl, :, :],
                                    op0=OP.add, op1=OP.max)
                            else:  # [F; O]
                                nc.scalar.activation(o_t[hsl, :, :], ptt[64:128],
                                                     AF.Tanh, scale=0.5,
                                                     bias=bias_sb[64:128, 2:3])

                    # ---- gate prep on DVE (f_t, o_t hold tanh halves:
                    # F = 0.5*f_t + 0.5, O = 0.5*o_t + 0.5)
                    g_t = scanp.tile([128, N, 1 + SC], FP, tag="g_t")
                    nc.gpsimd.memset(g_t[:, :, 0:1], 0.0)
                    nc.vector.tensor_scalar(g_t[:, :, 1:], f_t[:, :, :], -0.5, 0.5,
                                            op0=OP.mult, op1=OP.add)  # G = 1 - F
                    # F = 0.5*th + 0.5 (reconstruct in place on DVE)
                    nc.vector.tensor_scalar(f_t[:, :, :], f_t[:, :, :], 0.5, 0.5,
                                            op0=OP.mult, op1=OP.add)
                    # O = 0.5*th + 0.5 (reconstruct on GpSimd)
                    nc.gpsimd.tensor_scalar(o_t[:, :, :], o_t[:, :, :], 0.5, 0.5,
                                            op0=OP.mult, op1=OP.add)
                    fz = scanp.tile([128, N, 1 + SC], FP, tag="fz")
                    nc.vector.scalar_tensor_tensor(fz[:, :, 1:], q_t[:, :, :], -1.0,
                                                   f_t[:, :, :], op0=OP.add, op1=OP.mult)
                    # carry into the reset column
                    if ci == 0:
                        nc.vector.tensor_copy(fz[:, :, 0], hid_sb[:, :])
                    else:
                        nc.vector.tensor_copy(fz[:, :, 0], prev_c[:, :, SC])

                    # ---- the recurrence: one hardware scan over the whole chunk
                    c_t = scanp.tile([128, N, 1 + SC], FP, tag="c_t")
                    nc.vector.tensor_tensor_scan(
                        c_t.rearrange("p n s -> p (n s)"),
                        g_t.rearrange("p n s -> p (n s)"),
                        fz.rearrange("p n s -> p (n s)"),
                        0.0, op0=OP.mult, op1=OP.add,
                    )
                    prev_c = c_t

                    # ---- Hout = O * C, written in (s, n) layout for the DMA out
                    ho_t = actp.tile([128, SC, N], FP, tag="ho_t")
                    nc.gpsimd.tensor_tensor(ho_t.transpose([0, 2, 1]), o_t[:, :, :],
                                            c_t[:, :, 1:], op=OP.mult)
                    hdst = hout[s0:s0 + SC, :, :, :].transpose([1, 2, 0, 3]) \
                        .rearrange("b h s n -> (b h) s n")
                    nc.sync.dma_start(hdst, ho_t[:, :, :])

                cl_t = actp.tile([128, N], FP, tag="cl_t", bufs=1)
                nc.vector.tensor_copy(cl_t[:, :], prev_c[:, :, SC])
                nc.sync.dma_start(clast[:, :], cl_t[:, :])

    nc.compile()
    return nc


def _mm_np_dtype():
    return mybir.dt.np(MM_DT)


def pack_weights(W: np.ndarray) -> np.ndarray:
    """W [192, 64, 1, 9] -> lhsT tiles [2, 5, 128, 128] (group, tap-pair, K, M).

    Group 0 covers output channels 0:128 ([Z;F]), group 1 covers 64:192
    ([F;O]).  K rows 0:64 hold tap 2p, rows 64:128 hold tap 2p+1 (zeros for
    the nonexistent tap 9)."""
    Wt = np.asarray(W, dtype=np.float32)[:, :, 0, :]  # [192, 64, 9]
    Wl = np.zeros((2, 5, 128, 128), dtype=np.float32)
    for grp, ch0 in ((0, 0), (1, 64)):
        for p in range(5):
            Wl[grp, p, 0:64, :] = Wt[ch0:ch0 + 128, :, 2 * p].T
            if 2 * p + 1 <= 8:
                Wl[grp, p, 64:128, :] = Wt[ch0:ch0 + 128, :, 2 * p + 1].T
    return Wl.astype(_mm_np_dtype())


def pack_bias(b: np.ndarray) -> np.ndarray:
    b = np.asarray(b, dtype=np.float32)
    bias_t = np.zeros((128, 4), dtype=np.float32)
    hh = np.arange(128) % 64
    bias_t[:, 0] = b[hh]                 # z bias (exp pass, partitions = h)
    bias_t[:, 1] = 0.5 * b[64 + hh]      # f bias (tanh half-scale)
    bias_t[:, 2] = 0.5 * b[128 + hh]     # o bias (tanh half-scale)
    bias_t[:, 3] = b[hh] + 1.0           # z bias + 1 (elu max identity)
    return bias_t


def pad_x(input_data: np.ndarray) -> np.ndarray:
    """[S, B, C, N] fp32 -> [2C, S, B, NP_PAD] in MM_DT: rows 0:C hold the
    shift-4 halo layout (taps 2p), rows C:2C the shift-3 layout (taps 2p+1)."""
    s_total, b_, c_, n_ = input_data.shape
    xt = np.asarray(input_data).astype(_mm_np_dtype()).transpose(2, 0, 1, 3)
    xab = np.zeros((2 * c_, s_total, b_, NP_PAD), dtype=_mm_np_dtype())
    xab[0:c_, :, :, 4:4 + n_] = xt
    xab[c_:, :, :, 3:3 + n_] = xt
    return xab


_NC_CACHE = {}


def build_in_maps(inputs: dict) -> list[dict]:
    input_data = np.asarray(inputs["input_data"], dtype=np.float32)
    hidden = np.asarray(inputs["hidden"], dtype=np.float32)
    xm = pad_x(input_data)
    Wl = pack_weights(inputs["W"])
    bias_t = pack_bias(inputs["b"])
    in_maps = []
    for i in range(NCORES):
        bsl = slice(i * B_LOC, (i + 1) * B_LOC)
        in_maps.append({
            "xm": np.ascontiguousarray(xm[:, :, bsl]),
            "bias": bias_t,
            "wl": Wl,
            "h0": np.ascontiguousarray(hidden[bsl]).reshape(128, N),
        })
    return in_maps


def kernel(input_data, hidden, W, b):
    global LAST_RESULTS
    from concourse.bass_utils import run_bass_kernel_spmd

    s_total = np.asarray(input_data).shape[0]
    if s_total not in _NC_CACHE:
        _NC_CACHE[s_total] = build_program(s_total)
    nc = _NC_CACHE[s_total]

    in_maps = build_in_maps({
        "input_data": input_data, "hidden": hidden, "W": W, "b": b,
    })
    res = run_bass_kernel_spmd(nc, in_maps, list(range(NCORES)))
    LAST_RESULTS = res

    houts = [res.results[i]["hout"] for i in range(NCORES)]
    clasts = [res.results[i]["clast"].reshape(B_LOC, H, N) for i in range(NCORES)]
    Hout = np.concatenate(houts, axis=1)
    C_last = np.concatenate(clasts, axis=0)[None]
    return Hout, C_last


# revision 35
# speedup vs baseline: 4.8965x; 4.4061x over previous
"""Trainium2 Bass kernel for nn_CQRNLayer (quasi-recurrent conv layer).

Computation (per the reference):
  Y = Conv2d(C=64 -> 3H=192, kernel (1,9), pad (0,4)) over (S*B, C, 1, N) + bias
  Z, F, O = split(Y); Z = elu(Z); F = sigmoid(F); O = sigmoid(O)
  C_t = F_t * Z_t + (1 - F_t) * C_{t-1}   (scan over S, C_{-1} = hidden)
  Hout = O * C_seq ; outputs (Hout, C_seq[-1:])

Sharding: data-parallel over batch (B=16 -> 2 per core on 8 cores).

Kernel strategy (per core):
  * Conv as K=128, M=128, free=512 matmuls: the 9 taps are packed in pairs
    along the contraction dim.  SBUF holds X twice: partitions 0:64 with the
    n-axis halo-padded (shift 4), partitions 64:128 shifted by 3; a tap pair
    (2p, 2p+1) then reads one shifted window of both halves.  Tap 8 rides a
    zero-padded second half of the weights.  The 192 output channels are
    covered by two M=128 groups: [Z;F] (ch 0:128) and [F;O] (ch 64:192,
    F computed twice) - uniform full-array matmuls, float32r at full PE rate.
  * Weights pre-packed on host to [2, 5, 128, 128] (group, pair, K, M).
  * ScalarE evacuates PSUM applying exp/sigmoid (+conv bias) directly into
    an s-innermost layout [128=(b,h), n, s], using per-b partition-shifted
    passes.
  * elu(z)+1 = max(z + 1, min(exp(z), 1)) -> DVE scalar_tensor_tensor reading
    the conv PSUM directly (no relu pass).
  * The recurrence runs on the DVE hardware scan (tensor_tensor_scan) along
    the free dim: chains of length SC per n, separated by reset columns
    (G=0, FZ=carry) so one instruction scans a whole [128, n*(SC+1)] tile.
  * Hout = O * C on GpSimd; DMA out per chunk.
"""

import os
import sys

import numpy as np

for _p in ("/opt/trn_rl_repo", "/root/.axon_site/_ro/trn_rl_repo"):
    if os.path.isdir(_p) and _p not in sys.path:
        sys.path.append(_p)

import concourse.bass as bass
import concourse.bacc as bacc
import concourse.mybir as mybir
import concourse.tile as tile

FP = mybir.dt.float32
AF = mybir.ActivationFunctionType
OP = mybir.AluOpType

S, B, C, N, H = 256, 16, 64, 128, 64
NCORES = 8
B_LOC = B // NCORES  # 2
SC = 16              # seq-len macro chunk
NP_PAD = 136         # 128 + 2*4 halo for the 9-tap conv
NP_IN = NP_PAD + 1   # host-padded n extent (both shifted reads from one array)

MM_DT = mybir.dt.bfloat16  # conv matmul operand dtype (full PE rate)

LAST_RESULTS = None  # BassKernelResults of the most recent run (for test.py)


def build_program(s_total: int = S, reps: int = 1):
    nchunks = s_total // SC
    nc = bacc.Bacc(
        "TRN2", target_bir_lowering=False, debug=False, num_devices=NCORES
    )

    xm = nc.declare_dram_parameter("xm", [2 * C, s_total, B_LOC, NP_PAD], MM_DT, isOutput=False)
    wl = nc.declare_dram_parameter("wl", [2, 5, 128, 128], MM_DT, isOutput=False)
    bias = nc.declare_dram_parameter("bias", [128, 5], FP, isOutput=False)
    h0 = nc.declare_dram_parameter("h0", [128, N], FP, isOutput=False)
    hout = nc.declare_dram_parameter("hout", [s_total, B_LOC, H, N], FP, isOutput=True)
    clast = nc.declare_dram_parameter("clast", [128, N], FP, isOutput=True)

    with tile.TileContext(nc) as tc:
        with (
            tc.tile_pool(name="constp", bufs=1) as constp,
            tc.tile_pool(name="x2p", bufs=2) as x2p,
            tc.tile_pool(name="actp", bufs=2) as actp,
            tc.tile_pool(name="scanp", bufs=2) as scanp,
            tc.tile_pool(name="psump", bufs=2, space="PSUM") as psump,
        ):
            w_sb = constp.tile([128, 2, 5, 128], MM_DT)
            nc.sync.dma_start(w_sb[:, :, :, :], wl[:, :, :, :].transpose([2, 0, 1, 3]))
            bias_sb = constp.tile([128, 5], FP)
            nc.sync.dma_start(bias_sb[:, :], bias[:, :])
            hid_sb = constp.tile([128, N], FP)
            nc.sync.dma_start(hid_sb[:, :], h0[:, :])

            for rep in range(reps):
                prev_c = None
                for ci in range(nchunks):
                    s0 = ci * SC
                    # ---- X staging: two n-shifted copies on the partition halves
                    # (host pre-shifts; each DMA is one long contiguous run per
                    # partition)
                    x2 = x2p.tile([128, SC, B_LOC, NP_PAD], MM_DT, tag="x2")
                    nc.sync.dma_start(x2[:, :, :, :], xm[:, s0:s0 + SC, :, :])

                    # ---- conv + evacuation, one (grp, b) unit at a time,
                    # cycling two 4-bank PSUM slots so the PE never idles.
                    # psum layout [128ch, s, n]; gate tiles [(b,h), n, s].
                    expz = actp.tile([128, N, SC], FP, tag="expz")
                    f_t = actp.tile([128, N, SC], FP, tag="f_t")
                    o_t = actp.tile([128, N, SC], FP, tag="o_t")
                    q_t = actp.tile([128, N, SC], FP, tag="q_t", bufs=1)
                    for b in range(B_LOC):
                        hsl = slice(b * 64, b * 64 + 64)
                        for grp in range(2):
                            pt = psump.tile([128, SC, N], FP, tag="ps", name=f"pt{grp}{b}")
                            for mu in range(SC // 4):
                                for p in range(5):
                                    nc.tensor.matmul(
                                        pt[:, mu * 4:(mu + 1) * 4, :],
                                        w_sb[:, grp, p, :],
                                        x2[:, mu * 4:(mu + 1) * 4, b, 2 * p:2 * p + 128],
                                        start=(p == 0),
                                        stop=(p == 4),
                                        skip_group_check=True,
                                    )
                            ptt = pt[:, :, :].transpose([0, 2, 1])  # [ch, n, s]
                            # sigmoid(x) = 0.5*tanh(x/2) + 0.5; tanh shares the
                            # ACT table with exp, so no act-table reloads.
                            if grp == 0:  # [Z; F]
                                nc.scalar.activation(expz[hsl, :, :], ptt[0:64],
                                                     AF.Exp, bias=bias_sb[0:64, 0:1])
                                nc.scalar.activation(f_t[hsl, :, :], ptt[64:128],
                                                     AF.Tanh, scale=0.5,
                                                     bias=bias_sb[64:128, 1:2])
                                # expz <- min(exp, 1) in place
                                nc.vector.tensor_scalar_min(expz[hsl, :, :],
                                                            expz[hsl, :, :], 1.0)
                                # q = elu(z)+1 = max(z + bz + 1, min(exp(z+bz), 1))
                                nc.vector.scalar_tensor_tensor(
                                    q_t[hsl, :, :], ptt[0:64],
                                    bias_sb[hsl, 3:4], expz[hsl, :, :],
                                    op0=OP.add, op1=OP.max)
                            else:  # [F; O]
                                nc.scalar.activation(o_t[hsl, :, :], ptt[64:128],
                                                     AF.Tanh, scale=0.5,
                                                     bias=bias_sb[64:128, 2:3])

                    # ---- gate prep on DVE (f_t, o_t hold tanh halves:
                    # F = 0.5*f_t + 0.5, O = 0.5*o_t + 0.5)
                    g_t = scanp.tile([128, N, 1 + SC], FP, tag="g_t")
                    nc.gpsimd.memset(g_t[:, :, 0:1], 0.0)
                    nc.vector.tensor_scalar(g_t[:, :, 1:], f_t[:, :, :], -0.5, 0.5,
                                            op0=OP.mult, op1=OP.add)  # G = 1 - F
                    # F = 0.5*th + 0.5 (reconstruct in place on ScalarE)
                    nc.scalar.activation(f_t[:, :, :], f_t[:, :, :], AF.Identity,
                                         scale=0.5, bias=bias_sb[:, 4:5])
                    # O = 0.5*th + 0.5 (reconstruct in place on GpSimd, contiguous)
                    nc.gpsimd.tensor_scalar(o_t[:, :, :], o_t[:, :, :], 0.5, 0.5,
                                            op0=OP.mult, op1=OP.add)
                    fz = scanp.tile([128, N, 1 + SC], FP, tag="fz")
                    nc.vector.scalar_tensor_tensor(fz[:, :, 1:], q_t[:, :, :], -1.0,
                                                   f_t[:, :, :], op0=OP.add, op1=OP.mult)
                    # carry into the reset column
                    if ci == 0:
                        nc.vector.tensor_copy(fz[:, :, 0], hid_sb[:, :])
                    else:
                        nc.vector.tensor_copy(fz[:, :, 0], prev_c[:, :, SC])

                    # ---- the recurrence: one hardware scan over the whole chunk
                    c_t = scanp.tile([128, N, 1 + SC], FP, tag="c_t")
                    nc.vector.tensor_tensor_scan(
                        c_t.rearrange("p n s -> p (n s)"),
                        g_t.rearrange("p n s -> p (n s)"),
                        fz.rearrange("p n s -> p (n s)"),
                        0.0, op0=OP.mult, op1=OP.add,
                    )
                    prev_c = c_t

                    # ---- Hout = O * C, written in (s, n) layout for the DMA out
                    ho_t = actp.tile([128, SC, N], FP, tag="ho_t")
                    nc.vector.tensor_tensor(ho_t.transpose([0, 2, 1]), o_t[:, :, :],
                                            c_t[:, :, 1:], op=OP.mult)
                    hdst = hout[s0:s0 + SC, :, :, :].transpose([1, 2, 0, 3]) \
                        .rearrange("b h s n -> (b h) s n")
                    nc.sync.dma_start(hdst, ho_t[:, :, :])

                cl_t = actp.tile([128, N], FP, tag="cl_t", bufs=1)
                nc.vector.tensor_copy(cl_t[:, :], prev_c[:, :, SC])
                nc.sync.dma_start(clast[:, :], cl_t[:, :])

    nc.compile()
    return nc


def _mm_np_dtype():
    return mybir.dt.np(MM_DT)


def pack_weights(W: np.ndarray) -> np.ndarray:
    """W [192, 64, 1, 9] -> lhsT tiles [2, 5, 128, 128] (group, tap-pair, K, M).

    Group 0 covers output channels 0:128 ([Z;F]), group 1 covers 64:192
    ([F;O]).  K rows 0:64 hold tap 2p, rows 64:128 hold tap 2p+1 (zeros for
    the nonexistent tap 9)."""
    Wt = np.asarray(W, dtype=np.float32)[:, :, 0, :]  # [192, 64, 9]
    Wl = np.zeros((2, 5, 128, 128), dtype=np.float32)
    for grp, ch0 in ((0, 0), (1, 64)):
        for p in range(5):
            Wl[grp, p, 0:64, :] = Wt[ch0:ch0 + 128, :, 2 * p].T
            if 2 * p + 1 <= 8:
                Wl[grp, p, 64:128, :] = Wt[ch0:ch0 + 128, :, 2 * p + 1].T
    return Wl.astype(_mm_np_dtype())


def pack_bias(b: np.ndarray) -> np.ndarray:
    b = np.asarray(b, dtype=np.float32)
    bias_t = np.zeros((128, 5), dtype=np.float32)
    hh = np.arange(128) % 64
    bias_t[:, 0] = b[hh]                 # z bias (exp pass, partitions = h)
    bias_t[:, 1] = 0.5 * b[64 + hh]      # f bias (tanh half-scale)
    bias_t[:, 2] = 0.5 * b[128 + hh]     # o bias (tanh half-scale)
    bias_t[:, 3] = b[hh] + 1.0           # z bias + 1 (elu max identity)
    bias_t[:, 4] = 0.5                   # tanh -> sigmoid reconstruction
    return bias_t


def pad_x(input_data: np.ndarray) -> np.ndarray:
    """[S, B, C, N] fp32 -> [2C, S, B, NP_PAD] in MM_DT: rows 0:C hold the
    shift-4 halo layout (taps 2p), rows C:2C the shift-3 layout (taps 2p+1)."""
    s_total, b_, c_, n_ = input_data.shape
    xt = np.asarray(input_data).astype(_mm_np_dtype()).transpose(2, 0, 1, 3)
    xab = np.zeros((2 * c_, s_total, b_, NP_PAD), dtype=_mm_np_dtype())
    xab[0:c_, :, :, 4:4 + n_] = xt
    xab[c_:, :, :, 3:3 + n_] = xt
    return xab


_NC_CACHE = {}


def build_in_maps(inputs: dict) -> list[dict]:
    input_data = np.asarray(inputs["input_data"], dtype=np.float32)
    hidden = np.asarray(inputs["hidden"], dtype=np.float32)
    xm = pad_x(input_data)
    Wl = pack_weights(inputs["W"])
    bias_t = pack_bias(inputs["b"])
    in_maps = []
    for i in range(NCORES):
        bsl = slice(i * B_LOC, (i + 1) * B_LOC)
        in_maps.append({
            "xm": np.ascontiguousarray(xm[:, :, bsl]),
            "bias": bias_t,
            "wl": Wl,
            "h0": np.ascontiguousarray(hidden[bsl]).reshape(128, N),
        })
    return in_maps


def kernel(input_data, hidden, W, b):
    global LAST_RESULTS
    from concourse.bass_utils import run_bass_kernel_spmd

    s_total = np.asarray(input_data).shape[0]
    if s_total not in _NC_CACHE:
        _NC_CACHE[s_total] = build_program(s_total)
    nc = _NC_CACHE[s_total]

    in_maps = build_in_maps({
        "input_data": input_data, "hidden": hidden, "W": W, "b": b,
    })
    res = run_bass_kernel_spmd(nc, in_maps, list(range(NCORES)))
    LAST_RESULTS = res

    houts = [res.results[i]["hout"] for i in range(NCORES)]
    clasts = [res.results[i]["clast"].reshape(B_LOC, H, N) for i in range(NCORES)]
    Hout = np.concatenate(houts, axis=1)
    C_last = np.concatenate(clasts, axis=0)[None]
    return Hout, C_last


# revision 38
# speedup vs baseline: 5.1541x; 1.0526x over previous
"""Trainium2 Bass kernel for nn_CQRNLayer (quasi-recurrent conv layer).

Computation (per the reference):
  Y = Conv2d(C=64 -> 3H=192, kernel (1,9), pad (0,4)) over (S*B, C, 1, N) + bias
  Z, F, O = split(Y); Z = elu(Z); F = sigmoid(F); O = sigmoid(O)
  C_t = F_t * Z_t + (1 - F_t) * C_{t-1}   (scan over S, C_{-1} = hidden)
  Hout = O * C_seq ; outputs (Hout, C_seq[-1:])

Sharding: data-parallel over batch (B=16 -> 2 per core on 8 cores).

Kernel strategy (per core):
  * Conv as K=128, M=128, free=512 matmuls: the 9 taps are packed in pairs
    along the contraction dim.  SBUF holds X twice: partitions 0:64 with the
    n-axis halo-padded (shift 4), partitions 64:128 shifted by 3; a tap pair
    (2p, 2p+1) then reads one shifted window of both halves.  Tap 8 rides a
    zero-padded second half of the weights.  The 192 output channels are
    covered by two M=128 groups: [Z;F] (ch 0:128) and [F;O] (ch 64:192,
    F computed twice) - uniform full-array matmuls, float32r at full PE rate.
  * Weights pre-packed on host to [2, 5, 128, 128] (group, pair, K, M).
  * ScalarE evacuates PSUM applying exp/sigmoid (+conv bias) directly into
    an s-innermost layout [128=(b,h), n, s], using per-b partition-shifted
    passes.
  * elu(z)+1 = max(z + 1, min(exp(z), 1)) -> DVE scalar_tensor_tensor reading
    the conv PSUM directly (no relu pass).
  * The recurrence runs on the DVE hardware scan (tensor_tensor_scan) along
    the free dim: chains of length SC per n, separated by reset columns
    (G=0, FZ=carry) so one instruction scans a whole [128, n*(SC+1)] tile.
  * Hout = O * C on GpSimd; DMA out per chunk.
"""

import os
import sys

import numpy as np

for _p in ("/opt/trn_rl_repo", "/root/.axon_site/_ro/trn_rl_repo"):
    if os.path.isdir(_p) and _p not in sys.path:
        sys.path.append(_p)

import concourse.bass as bass
import concourse.bacc as bacc
import concourse.mybir as mybir
import concourse.tile as tile

FP = mybir.dt.float32
AF = mybir.ActivationFunctionType
OP = mybir.AluOpType

S, B, C, N, H = 256, 16, 64, 128, 64
NCORES = 8
B_LOC = B // NCORES  # 2
SC = 16              # seq-len macro chunk
NP_PAD = 136         # 128 + 2*4 halo for the 9-tap conv
NP_IN = NP_PAD + 1   # host-padded n extent (both shifted reads from one array)

MM_DT = mybir.dt.bfloat16  # conv matmul operand dtype (full PE rate)
PSUM_SPLIT = 1             # psum tiles per (grp, b) per chunk (1 -> 4-bank tiles)

LAST_RESULTS = None  # BassKernelResults of the most recent run (for test.py)


def build_program(s_total: int = S, reps: int = 1):
    nchunks = s_total // SC
    nc = bacc.Bacc(
        "TRN2", target_bir_lowering=False, debug=False, num_devices=NCORES
    )

    xm = nc.declare_dram_parameter("xm", [2 * C, s_total, B_LOC, NP_PAD], MM_DT, isOutput=False)
    wl = nc.declare_dram_parameter("wl", [2, 5, 128, 128], MM_DT, isOutput=False)
    bias = nc.declare_dram_parameter("bias", [128, 5], FP, isOutput=False)
    h0 = nc.declare_dram_parameter("h0", [128, N], FP, isOutput=False)
    hout = nc.declare_dram_parameter("hout", [s_total, B_LOC, H, N], FP, isOutput=True)
    clast = nc.declare_dram_parameter("clast", [128, N], FP, isOutput=True)

    with tile.TileContext(nc) as tc:
        with (
            tc.tile_pool(name="constp", bufs=1) as constp,
            tc.tile_pool(name="x2p", bufs=2) as x2p,
            tc.tile_pool(name="actp", bufs=2) as actp,
            tc.tile_pool(name="scanp", bufs=2) as scanp,
            tc.tile_pool(name="psump", bufs=2, space="PSUM") as psump,
        ):
            w_sb = constp.tile([128, 2, 5, 128], MM_DT)
            nc.sync.dma_start(w_sb[:, :, :, :], wl[:, :, :, :].transpose([2, 0, 1, 3]))
            bias_sb = constp.tile([128, 5], FP)
            nc.sync.dma_start(bias_sb[:, :], bias[:, :])
            hid_sb = constp.tile([128, N], FP)
            nc.sync.dma_start(hid_sb[:, :], h0[:, :])

            for rep in range(reps):
                prev_c = None
                for ci in range(nchunks):
                    s0 = ci * SC
                    # ---- X staging: two n-shifted copies on the partition halves
                    # (host pre-shifts; each DMA is one long contiguous run per
                    # partition)
                    x2 = x2p.tile([128, SC, B_LOC, NP_PAD], MM_DT, tag="x2")
                    nc.sync.dma_start(x2[:, :, :, :], xm[:, s0:s0 + SC, :, :])

                    # ---- conv + evacuation, one (grp, b) unit at a time,
                    # cycling two 4-bank PSUM slots so the PE never idles.
                    # psum layout [128ch, s, n]; gate tiles [(b,h), n, s].
                    expz = actp.tile([128, N, SC], FP, tag="expz")
                    f_t = actp.tile([128, N, SC], FP, tag="f_t")
                    o_t = actp.tile([128, N, SC], FP, tag="o_t")
                    q_t = actp.tile([128, N, SC], FP, tag="q_t", bufs=1)
                    SH = SC // PSUM_SPLIT  # seq steps per psum tile
                    for b in range(B_LOC):
                        hsl = slice(b * 64, b * 64 + 64)
                        for grp in range(2):
                          for sh in range(PSUM_SPLIT):
                            ssl = slice(sh * SH, (sh + 1) * SH)
                            pt = psump.tile([128, SH, N], FP, tag="ps",
                                            name=f"pt{grp}{b}{sh}")
                            for mu in range(SH // 4):
                                for p in range(5):
                                    nc.tensor.matmul(
                                        pt[:, mu * 4:(mu + 1) * 4, :],
                                        w_sb[:, grp, p, :],
                                        x2[:, sh * SH + mu * 4:sh * SH + (mu + 1) * 4,
                                           b, 2 * p:2 * p + 128],
                                        start=(p == 0),
                                        stop=(p == 4),
                                        skip_group_check=True,
                                    )
                            ptt = pt[:, :, :].transpose([0, 2, 1])  # [ch, n, s]
                            # sigmoid(x) = 0.5*tanh(x/2) + 0.5; tanh shares the
                            # ACT table with exp, so no act-table reloads.
                            if grp == 0:  # [Z; F]
                                nc.scalar.activation(expz[hsl, :, ssl], ptt[0:64],
                                                     AF.Exp, bias=bias_sb[0:64, 0:1])
                                nc.scalar.activation(f_t[hsl, :, ssl], ptt[64:128],
                                                     AF.Tanh, scale=0.5,
                                                     bias=bias_sb[64:128, 1:2])
                                # expz <- min(exp, 1) in place
                                nc.vector.tensor_scalar_min(expz[hsl, :, ssl],
                                                            expz[hsl, :, ssl], 1.0)
                                # q = elu(z)+1 = max(z + bz + 1, min(exp(z+bz), 1))
                                nc.vector.scalar_tensor_tensor(
                                    q_t[hsl, :, ssl], ptt[0:64],
                                    bias_sb[hsl, 3:4], expz[hsl, :, ssl],
                                    op0=OP.add, op1=OP.max)
                            else:  # [F; O]
                                nc.scalar.activation(o_t[hsl, :, ssl], ptt[64:128],
                                                     AF.Tanh, scale=0.5,
                                                     bias=bias_sb[64:128, 2:3])

                    # ---- gate prep on DVE (f_t, o_t hold tanh halves:
                    # F = 0.5*f_t + 0.5, O = 0.5*o_t + 0.5)
                    g_t = scanp.tile([128, N, 1 + SC], FP, tag="g_t")
                    nc.gpsimd.memset(g_t[:, :, 0:1], 0.0)
                    nc.vector.tensor_scalar(g_t[:, :, 1:], f_t[:, :, :], -0.5, 0.5,
                                            op0=OP.mult, op1=OP.add)  # G = 1 - F
                    # F = 0.5*th + 0.5 (reconstruct in place on ScalarE)
                    nc.scalar.activation(f_t[:, :, :], f_t[:, :, :], AF.Identity,
                                         scale=0.5, bias=bias_sb[:, 4:5])
                    # O = 0.5*th + 0.5 (reconstruct in place on GpSimd, contiguous)
                    nc.gpsimd.tensor_scalar(o_t[:, :, :], o_t[:, :, :], 0.5, 0.5,
                                            op0=OP.mult, op1=OP.add)
                    fz = scanp.tile([128, N, 1 + SC], FP, tag="fz")
                    nc.vector.scalar_tensor_tensor(fz[:, :, 1:], q_t[:, :, :], -1.0,
                                                   f_t[:, :, :], op0=OP.add, op1=OP.mult)
                    # carry into the reset column
                    if ci == 0:
                        nc.vector.tensor_copy(fz[:, :, 0], hid_sb[:, :])
                    else:
                        nc.vector.tensor_copy(fz[:, :, 0], prev_c[:, :, SC])

                    # ---- the recurrence: one hardware scan over the whole chunk
                    c_t = scanp.tile([128, N, 1 + SC], FP, tag="c_t")
                    nc.vector.tensor_tensor_scan(
                        c_t.rearrange("p n s -> p (n s)"),
                        g_t.rearrange("p n s -> p (n s)"),
                        fz.rearrange("p n s -> p (n s)"),
                        0.0, op0=OP.mult, op1=OP.add,
                    )
                    prev_c = c_t

                    # ---- Hout = O * C, written in (s, n) layout for the DMA out
                    ho_t = actp.tile([128, SC, N], FP, tag="ho_t")
                    nc.vector.tensor_tensor(ho_t.transpose([0, 2, 1]), o_t[:, :, :],
                                            c_t[:, :, 1:], op=OP.mult)
                    hdst = hout[s0:s0 + SC, :, :, :].transpose([1, 2, 0, 3]) \
                        .rearrange("b h s n -> (b h) s n")
                    nc.sync.dma_start(hdst, ho_t[:, :, :])

                cl_t = actp.tile([128, N], FP, tag="cl_t", bufs=1)
                nc.vector.tensor_copy(cl_t[:, :], prev_c[:, :, SC])
                nc.sync.dma_start(clast[:, :], cl_t[:, :])

    nc.compile()
    return nc


def _mm_np_dtype():
    return mybir.dt.np(MM_DT)


def pack_weights(W: np.ndarray) -> np.ndarray:
    """W [192, 64, 1, 9] -> lhsT tiles [2, 5, 128, 128] (group, tap-pair, K, M).

    Group 0 covers output channels 0:128 ([Z;F]), group 1 covers 64:192
    ([F;O]).  K rows 0:64 hold tap 2p, rows 64:128 hold tap 2p+1 (zeros for
    the nonexistent tap 9)."""
    Wt = np.asarray(W, dtype=np.float32)[:, :, 0, :]  # [192, 64, 9]
    Wl = np.zeros((2, 5, 128, 128), dtype=np.float32)
    for grp, ch0 in ((0, 0), (1, 64)):
        for p in range(5):
            Wl[grp, p, 0:64, :] = Wt[ch0:ch0 + 128, :, 2 * p].T
            if 2 * p + 1 <= 8:
                Wl[grp, p, 64:128, :] = Wt[ch0:ch0 + 128, :, 2 * p + 1].T
    return Wl.astype(_mm_np_dtype())


def pack_bias(b: np.ndarray) -> np.ndarray:
    b = np.asarray(b, dtype=np.float32)
    bias_t = np.zeros((128, 5), dtype=np.float32)
    hh = np.arange(128) % 64
    bias_t[:, 0] = b[hh]                 # z bias (exp pass, partitions = h)
    bias_t[:, 1] = 0.5 * b[64 + hh]      # f bias (tanh half-scale)
    bias_t[:, 2] = 0.5 * b[128 + hh]     # o bias (tanh half-scale)
    bias_t[:, 3] = b[hh] + 1.0           # z bias + 1 (elu max identity)
    bias_t[:, 4] = 0.5                   # tanh -> sigmoid reconstruction
    return bias_t


def pad_x(input_data: np.ndarray) -> np.ndarray:
    """[S, B, C, N] fp32 -> [2C, S, B, NP_PAD] in MM_DT: rows 0:C hold the
    shift-4 halo layout (taps 2p), rows C:2C the shift-3 layout (taps 2p+1)."""
    s_total, b_, c_, n_ = input_data.shape
    xt = np.asarray(input_data).astype(_mm_np_dtype()).transpose(2, 0, 1, 3)
    xab = np.zeros((2 * c_, s_total, b_, NP_PAD), dtype=_mm_np_dtype())
    xab[0:c_, :, :, 4:4 + n_] = xt
    xab[c_:, :, :, 3:3 + n_] = xt
    return xab


_NC_CACHE = {}


def build_in_maps(inputs: dict) -> list[dict]:
    input_data = np.asarray(inputs["input_data"], dtype=np.float32)
    hidden = np.asarray(inputs["hidden"], dtype=np.float32)
    xm = pad_x(input_data)
    Wl = pack_weights(inputs["W"])
    bias_t = pack_bias(inputs["b"])
    in_maps = []
    for i in range(NCORES):
        bsl = slice(i * B_LOC, (i + 1) * B_LOC)
        in_maps.append({
            "xm": np.ascontiguousarray(xm[:, :, bsl]),
            "bias": bias_t,
            "wl": Wl,
            "h0": np.ascontiguousarray(hidden[bsl]).reshape(128, N),
        })
    return in_maps


def kernel(input_data, hidden, W, b):
    global LAST_RESULTS
    from concourse.bass_utils import run_bass_kernel_spmd

    s_total = np.asarray(input_data).shape[0]
    if s_total not in _NC_CACHE:
        _NC_CACHE[s_total] = build_program(s_total)
    nc = _NC_CACHE[s_total]

    in_maps = build_in_maps({
        "input_data": input_data, "hidden": hidden, "W": W, "b": b,
    })
    res = run_bass_kernel_spmd(nc, in_maps, list(range(NCORES)))
    LAST_RESULTS = res

    houts = [res.results[i]["hout"] for i in range(NCORES)]
    clasts = [res.results[i]["clast"].reshape(B_LOC, H, N) for i in range(NCORES)]
    Hout = np.concatenate(houts, axis=1)
    C_last = np.concatenate(clasts, axis=0)[None]
    return Hout, C_last
